# revision 45
# baseline (speedup 1.0000x reference)
"""MllamaTextCrossAttention kernel for 8 Trainium2 NeuronCores.

Strategy: tensor-parallel over heads (4 q-heads + 1 kv-head per core).
Each core computes q/k/v projections for its shard, fused QK-RMS-norm,
block-diagonal varlen attention (segments baked in at build time from the
actual cu_seqlen values), and a row-parallel o_proj partial of the full
[1024, 4096] output. The host sums the 8 partials.

All heavy matmuls run in fp16. Attention runs fully in transposed layout
(features on partitions): qT [d, tok], kT [d, kpos], scoresT [kpos, q],
attnT [d, q], so no device transposes are needed except v (PE-transpose
via identity). Softmax denominators / RMS statistics use ones-matmul
partition reductions; per-q broadcasts use K=1 outer-product matmuls.
Segment-boundary masking is folded into the exp activation as a
per-partition bias (-60 => exp ~ 0), so no zero-fill DMAs are needed.
"""
import os
import sys

if "/opt/trn_rl_repo" not in sys.path:
    sys.path.insert(0, "/opt/trn_rl_repo")

import numpy as np

HIDDEN = 4096
N_HEADS = 32
N_KV = 8
HD = 128
EPS = 1e-5
SCALE = HD ** -0.5
TQ = 1024
TK = 6404
TKP = 6656          # TK padded to 13*512
KTILES = TKP // 128  # 52
SLABS = TKP // 512   # 13
NCORES = 8
HPC = N_HEADS // NCORES  # 4 q-heads per core
P = 128
NC = HIDDEN // P     # 32 contraction chunks
MASKB = -60.0        # exp bias for masked rows
PARTIAL_TAIL = True  # split last-slab units' attention across slabs


def _segments(cu_q, cu_k):
    eq = [0] + [int(min(max(int(v), 0), TQ)) for v in cu_q] + [TQ]
    ek = [0] + [int(min(max(int(v), 0), TK)) for v in cu_k] + [TK]
    segs = []
    for i in range(len(eq) - 1):
        q0, q1 = eq[i], eq[i + 1]
        k0, k1 = ek[i], ek[i + 1]
        if q1 <= q0:
            continue
        if k1 <= k0:
            segs.append((q0, q1, 0, TK, True))   # empty kv -> uniform over Tk
        else:
            segs.append((q0, q1, k0, k1, False))
    return segs


def _mask_patterns(segs):
    """Distinct (lo, hi) row-validity patterns of boundary k-tiles.
    Column 0 is the all-valid pattern."""
    pats = [(0, P)]
    for (q0, q1, k0, k1, special) in segs:
        if special:
            continue
        t0, t1 = k0 // P, (k1 + P - 1) // P
        for t in range(t0, t1):
            lo = max(k0, t * P) - t * P
            hi = min(k1, (t + 1) * P) - t * P
            if (lo, hi) != (0, P) and (lo, hi) not in pats:
                pats.append((lo, hi))
    return pats


def _build(segs):
    import concourse.bass as bass
    import concourse.tile as tile
    from concourse import bacc, mybir

    F32 = mybir.dt.float32
    F32R = mybir.dt.float16
    AF = mybir.ActivationFunctionType
    MUL = mybir.AluOpType.mult

    pats = _mask_patterns(segs)
    NM = len(pats)
    pat_col = {p: i for i, p in enumerate(pats)}

    nc = bacc.Bacc("TRN2", target_bir_lowering=False, debug=False,
                   num_devices=NCORES)

    hT = nc.declare_dram_parameter("hT", [P, 2, NC, 512], F32R, isOutput=False)
    cT = nc.declare_dram_parameter("cT", [P, SLABS, NC, 512], F32R,
                                   isOutput=False)
    wqT = nc.declare_dram_parameter("wqT", [P, NC, P * HPC], F32R,
                                    isOutput=False)
    wkd = nc.declare_dram_parameter("wkd", [P, NC, P], F32R, isOutput=False)
    wvd = nc.declare_dram_parameter("wvd", [P, NC, P], F32R, isOutput=False)
    woT = nc.declare_dram_parameter("woT", [P, 8, HPC, 512], F32R,
                                    isOutput=False)
    wqk = nc.declare_dram_parameter("wqk", [1, P], F32R,
                                   isOutput=False)
    onec = nc.declare_dram_parameter("onec", [P, 1], F32R, isOutput=False)
    onec2 = nc.declare_dram_parameter("onec2", [P, 2], F32R, isOutput=False)
    oner = nc.declare_dram_parameter("oner", [1, P], F32R,
                                    isOutput=False)
    identd = nc.declare_dram_parameter("identd", [P, P], F32R, isOutput=False)
    zerosd = nc.declare_dram_parameter("zerosd", [P, 512], F32R,
                                       isOutput=False)
    onesd = nc.declare_dram_parameter("onesd", [P, 512], F32R, isOutput=False)
    maskd = nc.declare_dram_parameter("maskd", [P, NM], F32, isOutput=False)
    out = nc.declare_dram_parameter("o", [TQ, HIDDEN], F32R, isOutput=True)

    with tile.TileContext(nc) as tc:
        with tc.tile_pool(name="persist", bufs=1) as pp:
            qT = pp.tile([P, HPC, TQ], F32R)       # q transposed
            kT = pp.tile([P, KTILES, P], F32R)     # k transposed, 128-blocks
            vN = pp.tile([P, KTILES, P], F32R)     # v natural, 128-blocks
            aT = pp.tile([P, HPC, TQ], F32R)       # attn output transposed
            A_sb = pp.tile([P, KTILES], F32)       # 1/rms_k per kpos
            msum = pp.tile([P, KTILES], F32)       # sum k^2 per kpos
            mask_sb = pp.tile([P, NM], F32)        # exp bias mask columns
            onec_sb = pp.tile([P, 1], F32R)
            onec2_sb = pp.tile([P, 2], F32R)
            oner_sb = pp.tile([1, P], F32R)
            wqk_sb = pp.tile([1, P], F32R)
            ident_sb = pp.tile([P, P], F32R)
            eps_sb = pp.tile([P, 1], F32)
            nc.vector.memset(eps_sb[:], EPS)
            nc.gpsimd.dma_start(onec_sb[:], onec[:])
            nc.gpsimd.dma_start(onec2_sb[:], onec2[:])
            nc.gpsimd.dma_start(oner_sb[:], oner[:])
            nc.gpsimd.dma_start(wqk_sb[:], wqk[:])
            nc.gpsimd.dma_start(ident_sb[:], identd[:])
            nc.gpsimd.dma_start(mask_sb[:], maskd[:])

            seg_t01 = []
            for (q0, q1, k0, k1, special) in segs:
                seg_t01.append((k0 // P, (k1 + P - 1) // P))
            asb_hi = [None] * len(segs)

            def emit_asb(si, tb):
                # batched 1/rms_k for a tile range (keeps Exp tables resident
                # during the slab loop otherwise)
                t0, t1 = seg_t01[si]
                lo = asb_hi[si] if asb_hi[si] is not None else t0
                tb = min(tb, t1)
                if lo >= tb:
                    return
                sq = kvp.tile([P, KTILES], F32, tag="sqseg", name="sqseg")
                nc.scalar.activation(sq[:, lo:tb], msum[:, lo:tb], AF.Sqrt,
                                     bias=eps_sb[:], scale=1.0 / HD)
                nc.vector.reciprocal(A_sb[:, lo:tb], sq[:, lo:tb])
                asb_hi[si] = tb

            stash = {}  # unit id -> (dnA, avA) SBUF partials

            def emit_attention(unit, part="full"):
                # part: "full" = whole k-range; "A" = tiles below the last
                # slab, accumulated into SBUF; "B" = last-slab tiles, combined
                # with the stashed partials.
                (q0, q1, k0, k1, special), qc0, si = unit
                t0 = k0 // P
                t1 = (k1 + P - 1) // P
                split = 4 * (SLABS - 1)
                if part == "A":
                    ta, tb = t0, min(split, t1)
                elif part == "B":
                    ta, tb = min(split, t1), t1
                else:
                    ta, tb = t0, t1
                nt = tb - ta
                nq = min(qc0 + P, q1) - qc0
                nf = HPC * nq
                if not special:
                    emit_asb(si, tb)
                psd = aps2.tile([1, HPC * P], F32, tag="small", name="psd")
                psa = aps2.tile([P, HPC, P], F32, tag="psa", name="psa")
                E = ep.tile([P, 14, HPC, P], F32R, tag="E", name="E")

                def av(ti):
                    nc.tensor.matmul(psa[:, :, 0:nq], vN[:, ta + ti, :],
                                     E[:, ti, :, 0:nq],
                                     start=(ti == 0), stop=(ti == nt - 1))

                for ti in range(nt):
                    t = ta + ti
                    if special:
                        lo = max(k0, t * P) - t * P
                        hi = min(k1, (t + 1) * P) - t * P
                        if lo > 0:
                            nc.sync.dma_start(E[0:lo, ti, :, 0:nq],
                                              zerosd[0:lo, :nf])
                        nc.sync.dma_start(E[lo:hi, ti, :, 0:nq],
                                          onesd[lo:hi, :nf])
                        if hi < P:
                            nc.sync.dma_start(E[hi:P, ti, :, 0:nq],
                                              zerosd[hi:P, :nf])
                        continue
                    lo = max(k0, t * P) - t * P
                    hi = min(k1, (t + 1) * P) - t * P
                    mc = pat_col[(lo, hi)]
                    pss = aps.tile([P, HPC, P], F32, tag="pss", name="pss")
                    nc.tensor.matmul(pss[:, :, 0:nq], kT[:, t, :],
                                     qT[:, :, qc0:qc0 + nq],
                                     start=True, stop=True)
                    nc.scalar.activation(E[:, ti, :, 0:nq], pss[:, :, 0:nq],
                                         AF.Exp,
                                         bias=mask_sb[:, mc:mc + 1],
                                         scale=A_sb[:, t:t + 1])
                for ti in range(nt):
                    av(ti)
                acc = ap.tile([P, HPC, P], F32R, tag="acc", name="acc")
                if nt == 1:
                    nc.vector.tensor_copy(acc[:, :, 0:nq], E[:, 0, :, 0:nq])
                else:
                    nc.vector.tensor_tensor(acc[:, :, 0:nq], E[:, 0, :, 0:nq],
                                            E[:, 1, :, 0:nq],
                                            mybir.AluOpType.add)
                    for ti in range(2, nt):
                        nc.vector.tensor_tensor(acc[:, :, 0:nq],
                                                acc[:, :, 0:nq],
                                                E[:, ti, :, 0:nq],
                                                mybir.AluOpType.add)
                nc.tensor.matmul(psd[:, 0:nf], onec_sb[:],
                                 acc[:, :, 0:nq], start=True, stop=True)
                if part == "A":
                    dnA = ap.tile([1, HPC * P], F32, tag="dnA", name="dnA")
                    nc.vector.tensor_copy(dnA[:, 0:nf], psd[:, 0:nf])
                    avA = ap.tile([P, HPC, P], F32R, tag="avA", name="avA")
                    nc.vector.tensor_copy(avA[:, :, 0:nq], psa[:, :, 0:nq])
                    stash[id(unit)] = (dnA, avA)
                    return
                if part == "B":
                    dnA, avA = stash.pop(id(unit))
                    dn = ap.tile([1, HPC * P], F32, tag="rden", name="dn")
                    nc.vector.tensor_tensor(dn[:, 0:nf], psd[:, 0:nf],
                                            dnA[:, 0:nf],
                                            mybir.AluOpType.add)
                    rden = dn
                    av = ap.tile([P, HPC, P], F32R, tag="av", name="av")
                    nc.vector.tensor_tensor(av[:, :, 0:nq], psa[:, :, 0:nq],
                                            avA[:, :, 0:nq],
                                            mybir.AluOpType.add)
                    avsrc = av
                else:
                    rden = ap.tile([1, HPC * P], F32, tag="rden", name="rden")
                    avsrc = psa
                nc.vector.reciprocal_approx_fast(rden[:, 0:nf],
                                                 rden[:, 0:nf] if part == "B"
                                                 else psd[:, 0:nf])
                rden16 = ap.tile([1, HPC * P], F32R, tag="rden16",
                                 name="rden16")
                nc.vector.tensor_copy(rden16[:, 0:nf], rden[:, 0:nf])
                psb = aps2.tile([P, HPC * P], F32, tag="small", name="psbA")
                nc.tensor.matmul(psb[:, 0:nf], oner_sb[:],
                                 rden16[:, 0:nf], start=True, stop=True)
                bden = ap.tile([P, HPC, P], F32R, tag="bden", name="bden")
                nc.vector.tensor_copy(bden[:, :, 0:nq], psb[:, 0:nf])
                for j in range(HPC):
                    nc.vector.tensor_tensor(aT[:, j, qc0:qc0 + nq],
                                            avsrc[:, j, 0:nq],
                                            bden[:, j, 0:nq], MUL)

            def emit_o(qtiles, tail=False):
                for qt in qtiles:
                    for n8 in range(8):
                        pso = ops.tile([P, 512], F32, tag="pso", name="pso")
                        for co in range(HPC):
                            nc.tensor.matmul(pso[:],
                                             aT[:, co, qt * P:(qt + 1) * P],
                                             wo_sb[:, n8, co, :],
                                             start=(co == 0),
                                             stop=(co == HPC - 1))
                        osb = op.tile([P, 512], F32R, tag="osb", name="osb")
                        if tail and n8 % 2 == 1:
                            nc.scalar.activation(osb[:], pso[:], AF.Copy)
                        else:
                            nc.vector.tensor_copy(osb[:], pso[:])
                        eng = nc.sync if (tail and n8 % 2 == 1) else nc.gpsimd
                        eng.dma_start(
                            out[qt * P:(qt + 1) * P,
                                n8 * 512:(n8 + 1) * 512],
                            osb[:])

            # ---- concurrent scopes: Q phase + interleaved KV/ATTN ----------
            with tc.tile_pool(name="kvw", bufs=1) as kvw, \
                 tc.tile_pool(name="kvp", bufs=2) as kvp, \
                 tc.tile_pool(name="ctp", bufs=11) as ctp, \
                 tc.tile_pool(name="ap", bufs=2) as ap, \
                 tc.tile_pool(name="ep", bufs=2) as ep, \
                 tc.tile_pool(name="kvps", bufs=2, space="PSUM") as kvps:

                # Phase Q (emitted first; PE runs it while cT slabs stream in)
                # Software-pipelined: wq/ht DMAs dispatched (from the scalar
                # queue) LOOKAHEAD steps ahead of the matmuls that use them.
                with tc.tile_pool(name="wqp", bufs=16) as wqp, \
                     tc.tile_pool(name="qp", bufs=6) as qp, \
                     tc.tile_pool(name="sqp", bufs=2) as sqp, \
                     tc.tile_pool(name="qps", bufs=1, space="PSUM") as qps, \
                     tc.tile_pool(name="qps2", bufs=1, space="PSUM") as qps2:
                  LOOK = 4
                  wqs = [None] * 16
                  hts = {}

                  def fetch(j):
                      half, qr = divmod(j, 16)
                      if half == 0:
                          wq_q = wqp.tile([P, 2, 512], F32R, tag="wqq",
                                          name="wqq")
                          nc.sync.dma_start(wq_q[:],
                                            wqT[:, qr * 2:(qr + 1) * 2, :])
                          wqs[qr] = wq_q
                      ht_q = qp.tile([P, 2, 512], F32R, tag="htq", name="htq")
                      nc.scalar.dma_start(ht_q[:],
                                          hT[:, half, qr * 2:(qr + 1) * 2, :])
                      hts[j] = ht_q

                  for j in range(LOOK):
                      fetch(j)
                  for half in range(2):
                    tsl = slice(half * 512, (half + 1) * 512)
                    psq = [qps.tile([P, 512], F32, tag=f"q{f}",
                                    name=f"psq{f}") for f in range(HPC)]
                    for qr in range(16):
                        j = half * 16 + qr
                        if j + LOOK < 32:
                            fetch(j + LOOK)
                        wq_q = wqs[qr]
                        ht_q = hts.pop(j)
                        for cc in range(2):
                            for f in range(HPC):
                                nc.tensor.matmul(
                                    psq[f][:],
                                    wq_q[:, cc, f * P:(f + 1) * P],
                                    ht_q[:, cc, :],
                                    start=(qr == 0 and cc == 0),
                                    stop=(qr == 15 and cc == 1))
                    for f in range(HPC):
                        qsq = ap.tile([P, 512], F32R, tag="qsq", name="qsq")
                        nc.vector.tensor_copy(qT[:, f, tsl], psq[f][:])
                        nc.vector.tensor_tensor(qsq[:], qT[:, f, tsl],
                                                qT[:, f, tsl], MUL)
                        pss = qps2.tile([1, 512], F32, tag="pss", name="pssq")
                        nc.tensor.matmul(pss[:], onec_sb[:], qsq[:],
                                         start=True, stop=True)
                        sq = sqp.tile([1, 512], F32, tag="sq", name="sq")
                        nc.scalar.activation(sq[:], pss[:], AF.Sqrt,
                                             bias=eps_sb[0:1], scale=1.0 / HD)
                        nc.vector.reciprocal_approx_fast(sq[:], sq[:])
                        sq16 = sqp.tile([1, 512], F32R, tag="sq16",
                                        name="sq16")
                        nc.vector.tensor_copy(sq16[:], sq[:])
                        psb = qps2.tile([P, 512], F32, tag="psb", name="psbq")
                        nc.tensor.matmul(psb[:], wqk_sb[:], sq16[:],
                                         start=True, stop=True)
                        nc.vector.tensor_tensor(qT[:, f, tsl], qT[:, f, tsl],
                                                psb[:], MUL)

                # interleaved KV slabs + attention for completed segments
                aps_cm = tc.tile_pool(name="aps", bufs=2, space="PSUM")
                aps = aps_cm.__enter__()
                aps2_cm = tc.tile_pool(name="aps2", bufs=1, space="PSUM")
                aps2 = aps2_cm.__enter__()
                op_cm = tc.tile_pool(name="op", bufs=4)
                op = op_cm.__enter__()
                ops_cm = tc.tile_pool(name="ops", bufs=2, space="PSUM")
                ops = ops_cm.__enter__()
                wop_cm = tc.tile_pool(name="wop", bufs=1)
                wop = wop_cm.__enter__()
                wo_sb = wop.tile([P, 8, HPC, 512], F32R)
                nc.sync.dma_start(wo_sb[:], woT[:])

                # q-tile -> covering units (for o_proj readiness)
                NQT = TQ // P
                qt_need = [0] * NQT
                qt_done = [0] * NQT
                o_done = [False] * NQT
                wk_sb = kvw.tile([P, NC, P], F32R)
                nc.sync.dma_start(wk_sb[:], wkd[:])
                wv_sb = kvw.tile([P, NC, P], F32R)
                nc.sync.dma_start(wv_sb[:], wvd[:])
                units = []
                for si, sg in enumerate(sorted(segs,
                                               key=lambda x: TK if x[4]
                                               else x[3])):
                    klim = TK if sg[4] else sg[3]
                    orig_si = segs.index(sg)
                    for qc0 in range(sg[0], sg[1], P):
                        units.append(((sg, qc0, orig_si), klim))
                for (sg, qc0, _), _k in units:
                    nq = min(qc0 + P, sg[1]) - qc0
                    for qt in range(qc0 // P, (qc0 + nq + P - 1) // P):
                        qt_need[qt] += 1
                ui = 0

                def drain_o(tail=False):
                    ready = [qt for qt in range(NQT)
                             if not o_done[qt] and qt_done[qt] == qt_need[qt]]
                    if ready:
                        for qt in ready:
                            o_done[qt] = True
                        emit_o(ready, tail)

                def finish_unit(u):
                    (sg, qc0, _si) = u
                    nq = min(qc0 + P, sg[1]) - qc0
                    for qt in range(qc0 // P, (qc0 + nq + P - 1) // P):
                        qt_done[qt] += 1

                for s in range(SLABS):
                    # last slab: only cols < TK - 12*512 = 260 are real data
                    NS = 512 if s < SLABS - 1 else TK - (SLABS - 1) * 512
                    NT = 4 if s < SLABS - 1 else 3  # used 128-tiles in slab
                    cts = []
                    for q4 in range(8):
                        ct_q = ctp.tile([P, 4, 512], F32R, tag="ctq",
                                        name="ctq")
                        nc.sync.dma_start(ct_q[:, :, 0:NS],
                                          cT[:, s, q4 * 4:(q4 + 1) * 4, 0:NS])
                        cts.append(ct_q)
                    psk = kvps.tile([P, 512], F32, tag="pskv", name="psk")
                    for q4 in range(8):
                        for cc in range(4):
                            c = q4 * 4 + cc
                            nc.tensor.matmul(psk[:, 0:NS], wk_sb[:, c, :],
                                             cts[q4][:, cc, 0:NS],
                                             start=(c == 0),
                                             stop=(c == NC - 1))
                    if NS == 512:
                        nc.vector.tensor_copy(kT[:, 4 * s:4 * s + 4, :],
                                              psk[:])
                    else:
                        nc.vector.tensor_copy(kT[:, 4 * s:4 * s + 2, :],
                                              psk[:, 0:256])
                        nc.vector.tensor_copy(kT[:, 4 * s + 2, 0:NS - 256],
                                              psk[:, 256:NS])
                        nc.vector.memset(kT[:, 4 * s + 2, NS - 256:P], 0.0)
                        nc.vector.memset(kT[:, 4 * s + 3, :], 0.0)
                    ksq = kvp.tile([P, 512], F32R, tag="ksq", name="ksq")
                    kslab = kT[:, 4 * s:4 * s + 4, :]
                    nc.vector.tensor_tensor(ksq[:], kslab, kslab, MUL)
                    psv = kvps.tile([P, 512], F32, tag="pskv", name="psv")
                    for q4 in range(8):
                        for cc in range(4):
                            c = q4 * 4 + cc
                            nc.tensor.matmul(psv[:, 0:NS], wv_sb[:, c, :],
                                             cts[q4][:, cc, 0:NS],
                                             start=(c == 0),
                                             stop=(c == NC - 1))
                    vstage = kvp.tile([P, 512], F32R, tag="vstage",
                                      name="vstage")
                    nc.vector.tensor_copy(vstage[:, 0:NS], psv[:, 0:NS])
                    if NS < 512:
                        nc.vector.memset(vstage[:, NS:512], 0.0)
                    for t in range(NT):
                        psr = aps2.tile([P, 2], F32, tag="small", name="psr")
                        nc.tensor.matmul(psr[:], ksq[:, t * P:(t + 1) * P],
                                         onec2_sb[:], start=True, stop=True)
                        nc.vector.tensor_copy(msum[:, 4 * s + t:4 * s + t + 1],
                                              psr[:, 0:1])
                        pst = ops.tile([P, P], F32R, tag="pso", name="pst")
                        nc.tensor.transpose(pst[:],
                                            vstage[:, t * P:(t + 1) * P],
                                            ident_sb[:])
                        nc.vector.tensor_copy(vN[:, 4 * s + t, :], pst[:])
                    kmax = (s + 1) * 512
                    if s == SLABS - 1:
                        kmax = TKP + 1
                    slabs_left = SLABS - 1 - s
                    nready = sum(1 for u in units[ui:] if u[1] <= kmax)
                    if slabs_left > 0:
                        budget = max(1, -(-nready // max(1, slabs_left)))
                    else:
                        budget = len(units)
                    emitted = 0
                    while ui < len(units) and emitted < budget and \
                            units[ui][1] <= kmax:
                        u = units[ui][0]
                        emit_attention(u, "B" if id(u) in stash else "full")
                        finish_unit(u)
                        ui += 1
                        emitted += 1
                        drain_o(tail=(s == SLABS - 1))
                    if PARTIAL_TAIL and s == SLABS - 2:
                        # pre-compute the below-last-slab part of the
                        # remaining units' attention (overlaps the last
                        # slab's K/V matmuls)
                        split = 4 * (SLABS - 1)
                        for (u, klim) in units[ui:]:
                            sg = u[0]
                            if not sg[4] and sg[2] // P < split:
                                emit_attention(u, "A")

                drain_o(tail=True)
                wop_cm.__exit__(None, None, None)
                ops_cm.__exit__(None, None, None)
                op_cm.__exit__(None, None, None)
                aps2_cm.__exit__(None, None, None)
                aps_cm.__exit__(None, None, None)

    nc.finalize()
    return nc


def _prepare(inputs):
    gi = {k: np.asarray(v) for k, v in inputs.items()}
    hs = np.ascontiguousarray(gi["hidden_states"], dtype=np.float16)
    cs = np.ascontiguousarray(gi["cross_attention_states"], dtype=np.float16)
    Wq = np.ascontiguousarray(gi["Wq"], dtype=np.float16)
    Wk = np.ascontiguousarray(gi["Wk"], dtype=np.float16)
    Wv = np.ascontiguousarray(gi["Wv"], dtype=np.float16)
    Wo = np.ascontiguousarray(gi["Wo"], dtype=np.float16)
    qw = np.asarray(gi["q_norm_w"], dtype=np.float32).reshape(-1)
    kw = np.asarray(gi["k_norm_w"], dtype=np.float32).reshape(-1)
    cu_q = np.asarray(gi["cu_seqlen_q"]).reshape(-1)
    cu_k = np.asarray(gi["cu_seqlen_k"]).reshape(-1)

    segs = _segments(cu_q, cu_k)
    nc = _build(segs)

    pats = _mask_patterns(segs)
    maskb = np.full((P, len(pats)), MASKB, np.float32)
    for i, (lo, hi) in enumerate(pats):
        maskb[lo:hi, i] = 0.0

    # packed layouts: partition-major with long contiguous per-partition runs
    hTd = np.ascontiguousarray(
        hs.T.reshape(NC, P, 2, 512).transpose(1, 2, 0, 3))   # [128,2,32,512]
    cTp = np.zeros((HIDDEN, TKP), np.float16)
    cTp[:, :TK] = cs.T
    cTd = np.ascontiguousarray(
        cTp.reshape(NC, P, SLABS, 512).transpose(1, 2, 0, 3))  # [128,13,32,512]
    wqkv = (qw * kw * SCALE).reshape(1, P).astype(np.float16)
    onec = np.ones((P, 1), np.float16)
    onec2 = np.ones((P, 2), np.float16)
    oner = np.ones((1, P), np.float16)
    ident = np.eye(P, dtype=np.float16)
    zeros = np.zeros((P, 512), np.float16)
    ones = np.ones((P, 512), np.float16)

    in_maps = []
    for c in range(NCORES):
        fsl = slice(c * P * HPC, (c + 1) * P * HPC)
        ksl = slice(c * P, (c + 1) * P)
        wq_d = np.ascontiguousarray(
            Wq[fsl, :].T.reshape(NC, P, P * HPC).transpose(1, 0, 2))
        wk_d = np.ascontiguousarray(
            Wk[ksl, :].T.reshape(NC, P, P).transpose(1, 0, 2))
        wv_d = np.ascontiguousarray(
            Wv[ksl, :].T.reshape(NC, P, P).transpose(1, 0, 2))
        wo_d = np.ascontiguousarray(
            Wo[:, fsl].T.reshape(HPC, P, 8, 512).transpose(1, 2, 0, 3))
        in_maps.append({
            "hT": hTd,
            "cT": cTd,
            "wqT": wq_d,
            "wkd": wk_d,
            "wvd": wv_d,
            "woT": wo_d,
            "wqk": wqkv,
            "onec": onec,
            "onec2": onec2,
            "oner": oner,
            "identd": ident,
            "zerosd": zeros,
            "onesd": ones,
            "maskd": maskb,
        })

    return nc, in_maps


def _reduce(results) -> np.ndarray:
    o = np.zeros((TQ, HIDDEN), np.float64)
    for c in range(NCORES):
        o += results[c]["o"].astype(np.float64)
    return o.astype(np.float32)


def kernel(**inputs) -> np.ndarray:
    from concourse.bass_utils import run_bass_kernel_spmd

    nc, in_maps = _prepare(inputs)
    r = run_bass_kernel_spmd(nc, in_maps, list(range(NCORES)))
    return _reduce(r.results)


# revision 46
# speedup vs baseline: 1.1611x; 1.1611x over previous
"""MllamaTextCrossAttention kernel for 8 Trainium2 NeuronCores.

Strategy: tensor-parallel over heads (4 q-heads + 1 kv-head per core).
Each core computes q/k/v projections for its shard, fused QK-RMS-norm,
block-diagonal varlen attention (segments baked in at build time from the
actual cu_seqlen values), and a row-parallel o_proj partial of the full
[1024, 4096] output. The host sums the 8 partials.

All heavy matmuls run in fp16. Attention runs fully in transposed layout
(features on partitions): qT [d, tok], kT [d, kpos], scoresT [kpos, q],
attnT [d, q], so no device transposes are needed except v (PE-transpose
via identity). Softmax denominators / RMS statistics use ones-matmul
partition reductions; per-q broadcasts use K=1 outer-product matmuls.
Segment-boundary masking is folded into the exp activation as a
per-partition bias (-60 => exp ~ 0), so no zero-fill DMAs are needed.
"""
import os
import sys

if "/opt/trn_rl_repo" not in sys.path:
    sys.path.insert(0, "/opt/trn_rl_repo")

import numpy as np

HIDDEN = 4096
N_HEADS = 32
N_KV = 8
HD = 128
EPS = 1e-5
SCALE = HD ** -0.5
TQ = 1024
TK = 6404
TKP = 6656          # TK padded to 13*512
KTILES = TKP // 128  # 52
SLABS = TKP // 512   # 13
NCORES = 8
HPC = N_HEADS // NCORES  # 4 q-heads per core
P = 128
NC = HIDDEN // P     # 32 contraction chunks
MASKB = -60.0        # exp bias for masked rows
PARTIAL_TAIL = True  # split last-slab units' attention across slabs


def _segments(cu_q, cu_k):
    eq = [0] + [int(min(max(int(v), 0), TQ)) for v in cu_q] + [TQ]
    ek = [0] + [int(min(max(int(v), 0), TK)) for v in cu_k] + [TK]
    segs = []
    for i in range(len(eq) - 1):
        q0, q1 = eq[i], eq[i + 1]
        k0, k1 = ek[i], ek[i + 1]
        if q1 <= q0:
            continue
        if k1 <= k0:
            segs.append((q0, q1, 0, TK, True))   # empty kv -> uniform over Tk
        else:
            segs.append((q0, q1, k0, k1, False))
    return segs


def _mask_patterns(segs):
    """Distinct (lo, hi) row-validity patterns of boundary k-tiles.
    Column 0 is the all-valid pattern."""
    pats = [(0, P)]
    for (q0, q1, k0, k1, special) in segs:
        if special:
            continue
        t0, t1 = k0 // P, (k1 + P - 1) // P
        for t in range(t0, t1):
            lo = max(k0, t * P) - t * P
            hi = min(k1, (t + 1) * P) - t * P
            if (lo, hi) != (0, P) and (lo, hi) not in pats:
                pats.append((lo, hi))
    return pats


def _build(segs):
    import concourse.bass as bass
    import concourse.tile as tile
    from concourse import bacc, mybir

    F32 = mybir.dt.float32
    F32R = mybir.dt.float16
    AF = mybir.ActivationFunctionType
    MUL = mybir.AluOpType.mult

    pats = _mask_patterns(segs)
    NM = len(pats)
    pat_col = {p: i for i, p in enumerate(pats)}

    nc = bacc.Bacc("TRN2", target_bir_lowering=False, debug=False,
                   num_devices=NCORES)

    hT = nc.declare_dram_parameter("hT", [P, 2, NC, 512], F32R, isOutput=False)
    cT = nc.declare_dram_parameter("cT", [P, SLABS, NC, 512], F32R,
                                   isOutput=False)
    wqT = nc.declare_dram_parameter("wqT", [P, NC, P * HPC], F32R,
                                    isOutput=False)
    wkd = nc.declare_dram_parameter("wkd", [P, NC, P], F32R, isOutput=False)
    wvd = nc.declare_dram_parameter("wvd", [P, NC, P], F32R, isOutput=False)
    woT = nc.declare_dram_parameter("woT", [P, 8, HPC, 512], F32R,
                                    isOutput=False)
    wqk = nc.declare_dram_parameter("wqk", [1, P], F32R,
                                   isOutput=False)
    onec = nc.declare_dram_parameter("onec", [P, 1], F32R, isOutput=False)
    onec2 = nc.declare_dram_parameter("onec2", [P, 2], F32R, isOutput=False)
    oner = nc.declare_dram_parameter("oner", [1, P], F32R,
                                    isOutput=False)
    identd = nc.declare_dram_parameter("identd", [P, P], F32R, isOutput=False)
    zerosd = nc.declare_dram_parameter("zerosd", [P, 512], F32R,
                                       isOutput=False)
    onesd = nc.declare_dram_parameter("onesd", [P, 512], F32R, isOutput=False)
    maskd = nc.declare_dram_parameter("maskd", [P, NM], F32, isOutput=False)
    out = nc.declare_dram_parameter("o", [TQ, HIDDEN], F32R, isOutput=True)

    with tile.TileContext(nc) as tc:
        with tc.tile_pool(name="persist", bufs=1) as pp:
            qT = pp.tile([P, HPC, TQ], F32R)       # q transposed
            kT = pp.tile([P, KTILES, P], F32R)     # k transposed, 128-blocks
            vN = pp.tile([P, KTILES, P], F32R)     # v natural, 128-blocks
            aT = pp.tile([P, HPC, TQ], F32R)       # attn output transposed
            A_sb = pp.tile([P, KTILES], F32)       # 1/rms_k per kpos
            msum = pp.tile([P, KTILES], F32)       # sum k^2 per kpos
            mask_sb = pp.tile([P, NM], F32)        # exp bias mask columns
            onec_sb = pp.tile([P, 1], F32R)
            onec2_sb = pp.tile([P, 2], F32R)
            oner_sb = pp.tile([1, P], F32R)
            wqk_sb = pp.tile([1, P], F32R)
            ident_sb = pp.tile([P, P], F32R)
            eps_sb = pp.tile([P, 1], F32)
            nc.vector.memset(eps_sb[:], EPS)
            nc.gpsimd.dma_start(onec_sb[:], onec[:])
            nc.gpsimd.dma_start(onec2_sb[:], onec2[:])
            nc.gpsimd.dma_start(oner_sb[:], oner[:])
            nc.gpsimd.dma_start(wqk_sb[:], wqk[:])
            nc.gpsimd.dma_start(ident_sb[:], identd[:])
            nc.gpsimd.dma_start(mask_sb[:], maskd[:])

            seg_t01 = []
            for (q0, q1, k0, k1, special) in segs:
                seg_t01.append((k0 // P, (k1 + P - 1) // P))
            asb_hi = [None] * len(segs)

            def emit_asb(si, tb):
                # batched 1/rms_k for a tile range (keeps Exp tables resident
                # during the slab loop otherwise)
                t0, t1 = seg_t01[si]
                lo = asb_hi[si] if asb_hi[si] is not None else t0
                tb = min(tb, t1)
                if lo >= tb:
                    return
                sq = kvp.tile([P, KTILES], F32, tag="sqseg", name="sqseg")
                nc.scalar.activation(sq[:, lo:tb], msum[:, lo:tb], AF.Sqrt,
                                     bias=eps_sb[:], scale=1.0 / HD)
                nc.vector.reciprocal(A_sb[:, lo:tb], sq[:, lo:tb])
                asb_hi[si] = tb

            stash = {}  # unit id -> (dnA, avA) SBUF partials

            def emit_attention(unit, part="full"):
                # part: "full" = whole k-range; "A" = tiles below the last
                # slab, accumulated into SBUF; "B" = last-slab tiles, combined
                # with the stashed partials.
                (q0, q1, k0, k1, special), qc0, si = unit
                t0 = k0 // P
                t1 = (k1 + P - 1) // P
                split = 4 * (SLABS - 1)
                if part == "A":
                    ta, tb = t0, min(split, t1)
                elif part == "B":
                    ta, tb = min(split, t1), t1
                else:
                    ta, tb = t0, t1
                nt = tb - ta
                nq = min(qc0 + P, q1) - qc0
                nf = HPC * nq
                if not special:
                    emit_asb(si, tb)
                psd = aps2.tile([1, HPC * P], F32, tag="small", name="psd")
                psa = aps2.tile([P, HPC, P], F32, tag="psa", name="psa")
                E = ep.tile([P, 14, HPC, P], F32R, tag="E", name="E")
                for ti in range(nt):
                    t = ta + ti
                    if special:
                        lo = max(k0, t * P) - t * P
                        hi = min(k1, (t + 1) * P) - t * P
                        if lo > 0:
                            nc.sync.dma_start(E[0:lo, ti, :, 0:nq],
                                              zerosd[0:lo, :nf])
                        nc.sync.dma_start(E[lo:hi, ti, :, 0:nq],
                                          onesd[lo:hi, :nf])
                        if hi < P:
                            nc.sync.dma_start(E[hi:P, ti, :, 0:nq],
                                              zerosd[hi:P, :nf])
                        continue
                    lo = max(k0, t * P) - t * P
                    hi = min(k1, (t + 1) * P) - t * P
                    mc = pat_col[(lo, hi)]
                    pss = aps.tile([P, HPC, P], F32, tag="pss", name="pss")
                    nc.tensor.matmul(pss[:, :, 0:nq], kT[:, t, :],
                                     qT[:, :, qc0:qc0 + nq],
                                     start=True, stop=True)
                    nc.scalar.activation(E[:, ti, :, 0:nq], pss[:, :, 0:nq],
                                         AF.Exp,
                                         bias=mask_sb[:, mc:mc + 1],
                                         scale=A_sb[:, t:t + 1])
                acc = ap.tile([P, HPC, P], F32R, tag="acc", name="acc")
                if nt == 1:
                    nc.vector.tensor_copy(acc[:, :, 0:nq], E[:, 0, :, 0:nq])
                else:
                    nc.vector.tensor_tensor(acc[:, :, 0:nq], E[:, 0, :, 0:nq],
                                            E[:, 1, :, 0:nq],
                                            mybir.AluOpType.add)
                    for ti in range(2, nt):
                        nc.vector.tensor_tensor(acc[:, :, 0:nq],
                                                acc[:, :, 0:nq],
                                                E[:, ti, :, 0:nq],
                                                mybir.AluOpType.add)
                nc.tensor.matmul(psd[:, 0:nf], onec_sb[:],
                                 acc[:, :, 0:nq], start=True, stop=True)
                for ti in range(nt):
                    nc.tensor.matmul(psa[:, :, 0:nq], vN[:, ta + ti, :],
                                     E[:, ti, :, 0:nq],
                                     start=(ti == 0), stop=(ti == nt - 1))
                if part == "A":
                    dnA = ap.tile([1, HPC * P], F32, tag="dnA", name="dnA")
                    nc.vector.tensor_copy(dnA[:, 0:nf], psd[:, 0:nf])
                    avA = ap.tile([P, HPC, P], F32R, tag="avA", name="avA")
                    nc.vector.tensor_copy(avA[:, :, 0:nq], psa[:, :, 0:nq])
                    stash[id(unit)] = (dnA, avA)
                    return
                if part == "B":
                    dnA, avA = stash.pop(id(unit))
                    dn = ap.tile([1, HPC * P], F32, tag="rden", name="dn")
                    nc.vector.tensor_tensor(dn[:, 0:nf], psd[:, 0:nf],
                                            dnA[:, 0:nf],
                                            mybir.AluOpType.add)
                    rden = dn
                    av = ap.tile([P, HPC, P], F32R, tag="av", name="av")
                    nc.vector.tensor_tensor(av[:, :, 0:nq], psa[:, :, 0:nq],
                                            avA[:, :, 0:nq],
                                            mybir.AluOpType.add)
                    avsrc = av
                else:
                    rden = ap.tile([1, HPC * P], F32, tag="rden", name="rden")
                    avsrc = psa
                nc.vector.reciprocal_approx_fast(rden[:, 0:nf],
                                                 rden[:, 0:nf] if part == "B"
                                                 else psd[:, 0:nf])
                rden16 = ap.tile([1, HPC * P], F32R, tag="rden16",
                                 name="rden16")
                nc.vector.tensor_copy(rden16[:, 0:nf], rden[:, 0:nf])
                psb = aps2.tile([P, HPC * P], F32, tag="small", name="psbA")
                nc.tensor.matmul(psb[:, 0:nf], oner_sb[:],
                                 rden16[:, 0:nf], start=True, stop=True)
                bden = ap.tile([P, HPC, P], F32R, tag="bden", name="bden")
                nc.vector.tensor_copy(bden[:, :, 0:nq], psb[:, 0:nf])
                for j in range(HPC):
                    nc.vector.tensor_tensor(aT[:, j, qc0:qc0 + nq],
                                            avsrc[:, j, 0:nq],
                                            bden[:, j, 0:nq], MUL)

            def emit_o(qtiles, tail=False):
                for qt in qtiles:
                    for n8 in range(8):
                        pso = ops.tile([P, 512], F32, tag="pso", name="pso")
                        for co in range(HPC):
                            nc.tensor.matmul(pso[:],
                                             aT[:, co, qt * P:(qt + 1) * P],
                                             wo_sb[:, n8, co, :],
                                             start=(co == 0),
                                             stop=(co == HPC - 1))
                        osb = op.tile([P, 512], F32R, tag="osb", name="osb")
                        if tail and n8 % 2 == 1:
                            nc.scalar.activation(osb[:], pso[:], AF.Copy)
                        else:
                            nc.vector.tensor_copy(osb[:], pso[:])
                        eng = nc.sync if (tail and n8 % 2 == 1) else nc.gpsimd
                        eng.dma_start(
                            out[qt * P:(qt + 1) * P,
                                n8 * 512:(n8 + 1) * 512],
                            osb[:])

            # ---- concurrent scopes: Q phase + interleaved KV/ATTN ----------
            with tc.tile_pool(name="kvw", bufs=1) as kvw, \
                 tc.tile_pool(name="kvp", bufs=2) as kvp, \
                 tc.tile_pool(name="ctp", bufs=11) as ctp, \
                 tc.tile_pool(name="ap", bufs=2) as ap, \
                 tc.tile_pool(name="ep", bufs=2) as ep, \
                 tc.tile_pool(name="kvps", bufs=2, space="PSUM") as kvps:

                # Phase Q (emitted first; PE runs it while cT slabs stream in)
                # Software-pipelined: wq/ht DMAs dispatched (from the scalar
                # queue) LOOKAHEAD steps ahead of the matmuls that use them.
                with tc.tile_pool(name="wqp", bufs=16) as wqp, \
                     tc.tile_pool(name="qp", bufs=6) as qp, \
                     tc.tile_pool(name="sqp", bufs=2) as sqp, \
                     tc.tile_pool(name="qps", bufs=1, space="PSUM") as qps, \
                     tc.tile_pool(name="qps2", bufs=1, space="PSUM") as qps2:
                  LOOK = 4
                  wqs = [None] * 16
                  hts = {}

                  def fetch(j):
                      half, qr = divmod(j, 16)
                      if half == 0:
                          wq_q = wqp.tile([P, 2, 512], F32R, tag="wqq",
                                          name="wqq")
                          nc.sync.dma_start(wq_q[:],
                                            wqT[:, qr * 2:(qr + 1) * 2, :])
                          wqs[qr] = wq_q
                      ht_q = qp.tile([P, 2, 512], F32R, tag="htq", name="htq")
                      nc.scalar.dma_start(ht_q[:],
                                          hT[:, half, qr * 2:(qr + 1) * 2, :])
                      hts[j] = ht_q

                  for j in range(LOOK):
                      fetch(j)
                  for half in range(2):
                    tsl = slice(half * 512, (half + 1) * 512)
                    psq = [qps.tile([P, 512], F32, tag=f"q{f}",
                                    name=f"psq{f}") for f in range(HPC)]
                    for qr in range(16):
                        j = half * 16 + qr
                        if j + LOOK < 32:
                            fetch(j + LOOK)
                        wq_q = wqs[qr]
                        ht_q = hts.pop(j)
                        for cc in range(2):
                            for f in range(HPC):
                                nc.tensor.matmul(
                                    psq[f][:],
                                    wq_q[:, cc, f * P:(f + 1) * P],
                                    ht_q[:, cc, :],
                                    start=(qr == 0 and cc == 0),
                                    stop=(qr == 15 and cc == 1))
                    for f in range(HPC):
                        qsq = ap.tile([P, 512], F32R, tag="qsq", name="qsq")
                        nc.vector.tensor_copy(qT[:, f, tsl], psq[f][:])
                        nc.vector.tensor_tensor(qsq[:], qT[:, f, tsl],
                                                qT[:, f, tsl], MUL)
                        pss = qps2.tile([1, 512], F32, tag="pss", name="pssq")
                        nc.tensor.matmul(pss[:], onec_sb[:], qsq[:],
                                         start=True, stop=True)
                        sq = sqp.tile([1, 512], F32, tag="sq", name="sq")
                        nc.scalar.activation(sq[:], pss[:], AF.Sqrt,
                                             bias=eps_sb[0:1], scale=1.0 / HD)
                        nc.vector.reciprocal_approx_fast(sq[:], sq[:])
                        sq16 = sqp.tile([1, 512], F32R, tag="sq16",
                                        name="sq16")
                        nc.vector.tensor_copy(sq16[:], sq[:])
                        psb = qps2.tile([P, 512], F32, tag="psb", name="psbq")
                        nc.tensor.matmul(psb[:], wqk_sb[:], sq16[:],
                                         start=True, stop=True)
                        nc.vector.tensor_tensor(qT[:, f, tsl], qT[:, f, tsl],
                                                psb[:], MUL)

                # interleaved KV slabs + attention for completed segments
                aps_cm = tc.tile_pool(name="aps", bufs=2, space="PSUM")
                aps = aps_cm.__enter__()
                aps2_cm = tc.tile_pool(name="aps2", bufs=1, space="PSUM")
                aps2 = aps2_cm.__enter__()
                op_cm = tc.tile_pool(name="op", bufs=4)
                op = op_cm.__enter__()
                ops_cm = tc.tile_pool(name="ops", bufs=2, space="PSUM")
                ops = ops_cm.__enter__()
                wop_cm = tc.tile_pool(name="wop", bufs=1)
                wop = wop_cm.__enter__()
                wo_sb = wop.tile([P, 8, HPC, 512], F32R)
                nc.sync.dma_start(wo_sb[:], woT[:])

                # q-tile -> covering units (for o_proj readiness)
                NQT = TQ // P
                qt_need = [0] * NQT
                qt_done = [0] * NQT
                o_done = [False] * NQT
                wk_sb = kvw.tile([P, NC, P], F32R)
                nc.sync.dma_start(wk_sb[:], wkd[:])
                wv_sb = kvw.tile([P, NC, P], F32R)
                nc.sync.dma_start(wv_sb[:], wvd[:])
                units = []
                for si, sg in enumerate(sorted(segs,
                                               key=lambda x: TK if x[4]
                                               else x[3])):
                    klim = TK if sg[4] else sg[3]
                    orig_si = segs.index(sg)
                    for qc0 in range(sg[0], sg[1], P):
                        units.append(((sg, qc0, orig_si), klim))
                for (sg, qc0, _), _k in units:
                    nq = min(qc0 + P, sg[1]) - qc0
                    for qt in range(qc0 // P, (qc0 + nq + P - 1) // P):
                        qt_need[qt] += 1
                ui = 0

                def drain_o(tail=False):
                    ready = [qt for qt in range(NQT)
                             if not o_done[qt] and qt_done[qt] == qt_need[qt]]
                    if ready:
                        for qt in ready:
                            o_done[qt] = True
                        emit_o(ready, tail)

                def finish_unit(u):
                    (sg, qc0, _si) = u
                    nq = min(qc0 + P, sg[1]) - qc0
                    for qt in range(qc0 // P, (qc0 + nq + P - 1) // P):
                        qt_done[qt] += 1

                for s in range(SLABS):
                    # last slab: only cols < TK - 12*512 = 260 are real data
                    NS = 512 if s < SLABS - 1 else TK - (SLABS - 1) * 512
                    NT = 4 if s < SLABS - 1 else 3  # used 128-tiles in slab
                    cts = []
                    for q4 in range(8):
                        ct_q = ctp.tile([P, 4, 512], F32R, tag="ctq",
                                        name="ctq")
                        nc.sync.dma_start(ct_q[:, :, 0:NS],
                                          cT[:, s, q4 * 4:(q4 + 1) * 4, 0:NS])
                        cts.append(ct_q)
                    psk = kvps.tile([P, 512], F32, tag="pskv", name="psk")
                    for q4 in range(8):
                        for cc in range(4):
                            c = q4 * 4 + cc
                            nc.tensor.matmul(psk[:, 0:NS], wk_sb[:, c, :],
                                             cts[q4][:, cc, 0:NS],
                                             start=(c == 0),
                                             stop=(c == NC - 1))
                    if NS == 512:
                        nc.vector.tensor_copy(kT[:, 4 * s:4 * s + 4, :],
                                              psk[:])
                    else:
                        nc.vector.tensor_copy(kT[:, 4 * s:4 * s + 2, :],
                                              psk[:, 0:256])
                        nc.vector.tensor_copy(kT[:, 4 * s + 2, 0:NS - 256],
                                              psk[:, 256:NS])
                        nc.vector.memset(kT[:, 4 * s + 2, NS - 256:P], 0.0)
                        nc.vector.memset(kT[:, 4 * s + 3, :], 0.0)
                    ksq = kvp.tile([P, 512], F32R, tag="ksq", name="ksq")
                    kslab = kT[:, 4 * s:4 * s + 4, :]
                    nc.vector.tensor_tensor(ksq[:], kslab, kslab, MUL)
                    psv = kvps.tile([P, 512], F32, tag="pskv", name="psv")
                    for q4 in range(8):
                        for cc in range(4):
                            c = q4 * 4 + cc
                            nc.tensor.matmul(psv[:, 0:NS], wv_sb[:, c, :],
                                             cts[q4][:, cc, 0:NS],
                                             start=(c == 0),
                                             stop=(c == NC - 1))
                    vstage = kvp.tile([P, 512], F32R, tag="vstage",
                                      name="vstage")
                    nc.vector.tensor_copy(vstage[:, 0:NS], psv[:, 0:NS])
                    if NS < 512:
                        nc.vector.memset(vstage[:, NS:512], 0.0)
                    for t in range(NT):
                        psr = aps2.tile([P, 2], F32, tag="small", name="psr")
                        nc.tensor.matmul(psr[:], ksq[:, t * P:(t + 1) * P],
                                         onec2_sb[:], start=True, stop=True)
                        nc.vector.tensor_copy(msum[:, 4 * s + t:4 * s + t + 1],
                                              psr[:, 0:1])
                        pst = ops.tile([P, P], F32R, tag="pso", name="pst")
                        nc.tensor.transpose(pst[:],
                                            vstage[:, t * P:(t + 1) * P],
                                            ident_sb[:])
                        nc.vector.tensor_copy(vN[:, 4 * s + t, :], pst[:])
                    kmax = (s + 1) * 512
                    if s == SLABS - 1:
                        kmax = TKP + 1
                    slabs_left = SLABS - 1 - s
                    nready = sum(1 for u in units[ui:] if u[1] <= kmax)
                    if slabs_left > 0:
                        budget = max(1, -(-nready // max(1, slabs_left)))
                    else:
                        budget = len(units)
                    emitted = 0
                    while ui < len(units) and emitted < budget and \
                            units[ui][1] <= kmax:
                        u = units[ui][0]
                        emit_attention(u, "B" if id(u) in stash else "full")
                        finish_unit(u)
                        ui += 1
                        emitted += 1
                        drain_o(tail=(s == SLABS - 1))
                    if PARTIAL_TAIL and s == SLABS - 2:
                        # pre-compute the below-last-slab part of the
                        # remaining units' attention (overlaps the last
                        # slab's K/V matmuls)
                        split = 4 * (SLABS - 1)
                        for (u, klim) in units[ui:]:
                            sg = u[0]
                            if not sg[4] and sg[2] // P < split:
                                emit_attention(u, "A")

                drain_o(tail=True)
                wop_cm.__exit__(None, None, None)
                ops_cm.__exit__(None, None, None)
                op_cm.__exit__(None, None, None)
                aps2_cm.__exit__(None, None, None)
                aps_cm.__exit__(None, None, None)

    nc.finalize()
    return nc


def _prepare(inputs):
    gi = {k: np.asarray(v) for k, v in inputs.items()}
    hs = np.ascontiguousarray(gi["hidden_states"], dtype=np.float16)
    cs = np.ascontiguousarray(gi["cross_attention_states"], dtype=np.float16)
    Wq = np.ascontiguousarray(gi["Wq"], dtype=np.float16)
    Wk = np.ascontiguousarray(gi["Wk"], dtype=np.float16)
    Wv = np.ascontiguousarray(gi["Wv"], dtype=np.float16)
    Wo = np.ascontiguousarray(gi["Wo"], dtype=np.float16)
    qw = np.asarray(gi["q_norm_w"], dtype=np.float32).reshape(-1)
    kw = np.asarray(gi["k_norm_w"], dtype=np.float32).reshape(-1)
    cu_q = np.asarray(gi["cu_seqlen_q"]).reshape(-1)
    cu_k = np.asarray(gi["cu_seqlen_k"]).reshape(-1)

    segs = _segments(cu_q, cu_k)
    nc = _build(segs)

    pats = _mask_patterns(segs)
    maskb = np.full((P, len(pats)), MASKB, np.float32)
    for i, (lo, hi) in enumerate(pats):
        maskb[lo:hi, i] = 0.0

    # packed layouts: partition-major with long contiguous per-partition runs
    hTd = np.ascontiguousarray(
        hs.T.reshape(NC, P, 2, 512).transpose(1, 2, 0, 3))   # [128,2,32,512]
    cTp = np.zeros((HIDDEN, TKP), np.float16)
    cTp[:, :TK] = cs.T
    cTd = np.ascontiguousarray(
        cTp.reshape(NC, P, SLABS, 512).transpose(1, 2, 0, 3))  # [128,13,32,512]
    wqkv = (qw * kw * SCALE).reshape(1, P).astype(np.float16)
    onec = np.ones((P, 1), np.float16)
    onec2 = np.ones((P, 2), np.float16)
    oner = np.ones((1, P), np.float16)
    ident = np.eye(P, dtype=np.float16)
    zeros = np.zeros((P, 512), np.float16)
    ones = np.ones((P, 512), np.float16)

    in_maps = []
    for c in range(NCORES):
        fsl = slice(c * P * HPC, (c + 1) * P * HPC)
        ksl = slice(c * P, (c + 1) * P)
        wq_d = np.ascontiguousarray(
            Wq[fsl, :].T.reshape(NC, P, P * HPC).transpose(1, 0, 2))
        wk_d = np.ascontiguousarray(
            Wk[ksl, :].T.reshape(NC, P, P).transpose(1, 0, 2))
        wv_d = np.ascontiguousarray(
            Wv[ksl, :].T.reshape(NC, P, P).transpose(1, 0, 2))
        wo_d = np.ascontiguousarray(
            Wo[:, fsl].T.reshape(HPC, P, 8, 512).transpose(1, 2, 0, 3))
        in_maps.append({
            "hT": hTd,
            "cT": cTd,
            "wqT": wq_d,
            "wkd": wk_d,
            "wvd": wv_d,
            "woT": wo_d,
            "wqk": wqkv,
            "onec": onec,
            "onec2": onec2,
            "oner": oner,
            "identd": ident,
            "zerosd": zeros,
            "onesd": ones,
            "maskd": maskb,
        })

    return nc, in_maps


def _reduce(results) -> np.ndarray:
    o = np.zeros((TQ, HIDDEN), np.float64)
    for c in range(NCORES):
        o += results[c]["o"].astype(np.float64)
    return o.astype(np.float32)


def kernel(**inputs) -> np.ndarray:
    from concourse.bass_utils import run_bass_kernel_spmd

    nc, in_maps = _prepare(inputs)
    r = run_bass_kernel_spmd(nc, in_maps, list(range(NCORES)))
    return _reduce(r.results)
